# revision 25
# baseline (speedup 1.0000x reference)
"""Trainium2 Bass kernel for nn_BitfieldLinear (vq_codebook).

Reference computation:
    idx   = codes & 0xFF            (basis row, 256 entries)
    r_q   = (codes >> 8) & 0xFFF
    sign  = bit20 ? -1 : +1
    scale = sign * tanh(r_q / 4095)
    W     = scale[:, None] * basis[idx]        # [8192, 4096]
    y     = x @ W.T                            # [128, 8192]

Key factorization (never materialize the 128MB W):
    Z = x @ basis.T                            # [128, 256]  tiny matmul
    y[b, j] = scale[j] * Z[b, idx[j]]          # column gather + scale

The gather+scale is a matmul with a scaled one-hot matrix:
    G[k, j] = scale[j] * (idx[j] == k)         # [256, 1024] per core
    y_core  = Z @ G                            # [128, 1024]

Sharding: out_features column-parallel across 8 cores (1024 codes per
core); x and basis replicated (8-core collectives are ~68us on this
harness -- far slower than recomputing Z per core).

v4 layout (from v2/v3 trace analysis):
  - 9 input DMAs on the two HWDGE queues only (c128 first), sized so
    the last chunks are small: SDMA round-robins between queues at
    packet granularity, so all in-flight transfers complete near the
    stream end -- big early chunks + small late chunks minimize the
    Z-tail wait.
  - Z accumulation is commutative: k-tiles are consumed in DMA-arrival
    order, not index order.
  - PE HAM warmup (~16 dummy matmuls on a memset scratch): the PE clock
    gate needs ~3.4us of sustained activity before it doubles to
    2.4GHz; v3 warmed too briefly and ran all of Z at half clock.
  - G^T tiles (partition=code, scale fused into the is_eq build) are
    transposed into matmul layout by 16 xbar DMA-transposes on the idle
    HWDGE queues -- zero PE cost vs 16 PE transposes (BITF_GT=pe
    fallback keeps the old path).
  - copies split DVE/ACT (the ACT table load overlaps the stream).
QUANT="fp8" streams x/basis as fp8e3m4 (pre-scaled by 2/64, compensated
in the tanh coeffs): 1.5MB/core stream at ~1.4% rel err.
"""

import os
import sys

for _p in ("/opt/trn_rl_repo", "/opt/pypackages"):
    if _p not in sys.path:
        sys.path.insert(0, _p)

import numpy as np

import concourse.bacc as bacc
import concourse.mybir as mybir
import concourse.tile as tile
from concourse.alu_op_type import AluOpType
from concourse.bass_utils import run_bass_kernel_spmd

N_CORES = 8
BATCH = 128
IN_F = 4096
OUT_F = 8192
BASIS = 256
OPC = OUT_F // N_CORES      # 1024 output columns per core
NK = IN_F // 128            # 32 K-tiles
NT = OPC // 128             # 8 code-tiles per core
R_LEVELS = 4095.0

F32 = mybir.dt.float32
BF16 = mybir.dt.bfloat16
FP16 = mybir.dt.float16
FP8 = mybir.dt.float8e3
I32 = mybir.dt.int32

QUANT = os.environ.get("BITF_QUANT", "fp8")
GT_MODE = os.environ.get("BITF_GT", "pe")    # 'dma' xbar | 'pe' transpose
N_WARM = int(os.environ.get("BITF_WARM", "0"))
X_DT = FP8 if QUANT == "fp8" else FP16
B_DT = FP16 if QUANT == "fp16" else FP8
X_SCALE = 2.0 if QUANT == "fp8" else 1.0     # keep fp8e3m4 out of denormals
B_SCALE = 64.0 if QUANT in ("fp8", "fp8b") else 1.0
_COMP = 1.0 / (X_SCALE * B_SCALE)            # folded into tanh coeffs

# tanh(r) ~= r*(c0 + c1 u + c2 u^2 + c3 u^3), u=r^2, r in [0,1]
# (max rel err 8e-5, negligible vs the fp8 input error); coeffs carry
# the fp8 pre-scale compensation
TANH_C = [c * _COMP for c in (
    9.9991860534e-01, -3.3065536868e-01, 1.1890093882e-01,
    -2.6632289374e-02)]

# input chunks (k-tile ranges).  DMA completion sems fire ~1.5-2.3us
# after the data lands (HBM read receipt under load), so the first
# chunks are small to let Z start early, x is fully front-loaded on
# sync, and basis streams in 128KB chunks on scalar for a smooth
# availability ramp (v6's single k4-16 x chunk left the PE idle 2.3us,
# which also reset the HAM clock-gate warmup).
X_CHUNKS = [(0, 2), (2, 8), (8, 20), (20, 32)]       # 32+96+192+192 on sync
B_CHUNKS = [(0, 2), (2, 4), (4, 6), (6, 8), (8, 12), (12, 16),
            (16, 20), (20, 24), (24, 28), (28, 32)]
SYNC_ORDER = ["c128", "xc0", "xc1", "xc2", "xc3", "bc8", "bc9"]
SCAL_ORDER = ["bc0", "bc1", "bc2", "bc3", "bc4", "bc5", "bc6", "bc7"]
# basis arrives in ascending k on scalar at ~the cold-PE consumption
# rate; x (half the bytes per k-tile) stays ahead on sync
Z_ORDER = list(range(NK))


def build_nc():
    nc = bacc.Bacc(
        "TRN2",
        target_bir_lowering=False,
        debug=False,
        num_devices=N_CORES,
    )

    c128_d = nc.dram_tensor("c128", [128, NT], I32, kind="ExternalInput")
    xd = {
        f"xc{i}": nc.dram_tensor(f"xc{i}", [128, (e - s) * 128], X_DT,
                                 kind="ExternalInput")
        for i, (s, e) in enumerate(X_CHUNKS)
    }
    bd = {
        f"bc{i}": nc.dram_tensor(f"bc{i}", [128, (e - s) * 256], B_DT,
                                 kind="ExternalInput")
        for i, (s, e) in enumerate(B_CHUNKS)
    }
    out_d = nc.dram_tensor("out", [128, OPC], FP16, kind="ExternalOutput")

    with tile.TileContext(nc) as tc:
        with (
            tc.tile_pool(name="pool", bufs=1) as pool,
            tc.tile_pool(name="zps", bufs=1, space="PSUM") as zps,
            tc.tile_pool(name="tps", bufs=1, space="PSUM") as tps,
            tc.tile_pool(name="yps", bufs=2, space="PSUM") as yps,
        ):
            # (HAM warmup removed: measured on v3/v4, dummy matmuls
            # advance the clock-gate no faster than real Z matmuls do,
            # while delaying Z's start by their own duration)
            if N_WARM:
                scr = pool.tile([128, 256], BF16)
                nc.gpsimd.memset(scr[:], 0.0)
                for w in range(N_WARM):
                    wp = tps.tile([128, 256], F32, tag="warm",
                                  name=f"warm{w}")
                    nc.tensor.matmul(
                        wp[:], lhsT=scr[:, 0:128], rhs=scr[:],
                        start=True, stop=True,
                    )

            # ---- input DMAs: c128 on the otherwise-idle SWDGE queue
            # (so sync's first data chunk dispatches at t0), stream
            # chunks in k-order interleaved across the two HWDGE queues
            c128 = pool.tile([128, NT], I32)
            x_sb = pool.tile([128, IN_F], X_DT)
            b_sb = pool.tile([128, 2 * IN_F], B_DT)
            def issue(name):
                q = nc.sync if name in SYNC_ORDER else nc.scalar
                if name == "c128":
                    q.dma_start(out=c128[:], in_=c128_d[:])
                elif name.startswith("xc"):
                    s, e = X_CHUNKS[int(name[2:])]
                    q.dma_start(out=x_sb[:, s * 128:e * 128], in_=xd[name][:])
                else:
                    s, e = B_CHUNKS[int(name[2:])]
                    q.dma_start(out=b_sb[:, s * 256:e * 256], in_=bd[name][:])

            for name in ["c128", "xc0", "bc0", "bc1", "xc1", "bc2", "bc3",
                         "xc2", "bc4", "bc5", "xc3", "bc6", "bc7",
                         "bc8", "bc9"]:
                issue(name)

            # ---- constants: iota row [0..255], partition iota, identity
            iota_row_i = pool.tile([128, BASIS], I32)
            nc.gpsimd.iota(out=iota_row_i[:], pattern=[[1, BASIS]], base=0,
                           channel_multiplier=0)
            iota_part_i = pool.tile([128, 1], I32)
            nc.gpsimd.iota(out=iota_part_i[:], pattern=[[1, 1]], base=0,
                           channel_multiplier=1)

            # iota row in bf16: integer values <=255 are exact in bf16
            # and 2-byte in0 doubles DVE throughput on the G builds
            iota_b = pool.tile([128, BASIS], BF16)
            nc.vector.tensor_scalar_mul(out=iota_b[:], in0=iota_row_i[:],
                                        scalar1=1.0)
            iota_part_f = pool.tile([128, 1], F32)
            nc.vector.tensor_scalar_mul(out=iota_part_f[:],
                                        in0=iota_part_i[:], scalar1=1.0)
            identb = pool.tile([128, 128], BF16)
            nc.vector.tensor_scalar(
                out=identb[:], in0=iota_b[:, 0:128],
                scalar1=iota_part_f[:, 0:1], scalar2=None,
                op0=AluOpType.is_equal,
            )

            # ---- decode codes -> idx_f (f32), scl (f32), both [128, NT]
            idx_f = pool.tile([128, NT], F32)
            scl = pool.tile([128, NT], F32)

            idx_i = pool.tile([128, NT], I32, name="idx_i")
            nc.vector.tensor_scalar(
                out=idx_i[:], in0=c128[:],
                scalar1=255, scalar2=None, op0=AluOpType.bitwise_and,
            )
            nc.vector.tensor_scalar_mul(out=idx_f[:], in0=idx_i[:],
                                        scalar1=1.0)
            rq_i = pool.tile([128, NT], I32, name="rq_i")
            nc.vector.tensor_scalar(
                out=rq_i[:], in0=c128[:],
                scalar1=8, scalar2=4095,
                op0=AluOpType.logical_shift_right,
                op1=AluOpType.bitwise_and,
            )
            r = pool.tile([128, NT], F32, name="r")
            nc.vector.tensor_scalar_mul(out=r[:], in0=rq_i[:],
                                        scalar1=1.0 / R_LEVELS)
            u = pool.tile([128, NT], F32, name="u")
            nc.vector.tensor_tensor(out=u[:], in0=r[:], in1=r[:],
                                    op=AluOpType.mult)
            p = pool.tile([128, NT], F32, name="p")
            nc.vector.tensor_scalar(
                out=p[:], in0=u[:], scalar1=TANH_C[3], scalar2=TANH_C[2],
                op0=AluOpType.mult, op1=AluOpType.add,
            )
            for ci_ in (1, 0):
                nc.vector.tensor_tensor(out=p[:], in0=p[:], in1=u[:],
                                        op=AluOpType.mult)
                nc.vector.tensor_scalar(
                    out=p[:], in0=p[:], scalar1=TANH_C[ci_], scalar2=None,
                    op0=AluOpType.add,
                )
            th = pool.tile([128, NT], F32, name="th")
            nc.vector.tensor_tensor(out=th[:], in0=p[:], in1=r[:],
                                    op=AluOpType.mult)
            sg_i = pool.tile([128, NT], I32, name="sg_i")
            nc.vector.tensor_scalar(
                out=sg_i[:], in0=c128[:],
                scalar1=20, scalar2=1,
                op0=AluOpType.logical_shift_right,
                op1=AluOpType.bitwise_and,
            )
            sgn = pool.tile([128, NT], F32, name="sgn")
            nc.vector.tensor_scalar(
                out=sgn[:], in0=sg_i[:],
                scalar1=-2.0, scalar2=1.0,
                op0=AluOpType.mult, op1=AluOpType.add,
            )
            nc.vector.tensor_tensor(out=scl[:], in0=th[:], in1=sgn[:],
                                    op=AluOpType.mult)

            # ---- G^T tiles (bf16): gt[p, k] = scl[t*128+p] * (idx==k)
            gts = []
            for t in range(NT):
                gt = pool.tile([128, BASIS], BF16, tag=f"gt{t}",
                               name=f"gt{t}")
                nc.vector.tensor_scalar(
                    out=gt[:], in0=iota_b[:],
                    scalar1=idx_f[:, t:t + 1], scalar2=scl[:, t:t + 1],
                    op0=AluOpType.is_equal, op1=AluOpType.mult,
                )
                gts.append(gt)

            # ---- G in matmul layout [basis-k, code]: xbar DMA
            # transposes on the (by now idle) HWDGE queues, zero PE cost
            g_sb = [pool.tile([128, OPC], BF16, tag=f"g{h}", name=f"g_sb{h}")
                    for h in range(2)]
            if GT_MODE == "dma":
                for t in range(NT):
                    for h in range(2):
                        q = nc.sync if (t * 2 + h) % 2 == 0 else nc.scalar
                        q.dma_start(
                            out=g_sb[h][:, t * 128:(t + 1) * 128],
                            in_=gts[t][:, h * 128:(h + 1) * 128],
                            transpose=True,
                        )

            # The 8 G transposes of a bank write quarters of two shared
            # [128, 512] PSUM tiles so ONE wide copy per half moves them
            # to SBUF (v5/v6's per-tile copies gated the transposes at
            # ~650ns each).
            def emit_gt_bank(nch):
                tp = [tps.tile([128, 512], BF16, tag=f"gtp{h}",
                               name=f"gtp{h}_{nch}") for h in range(2)]
                for q, t in enumerate(range(nch * 4, nch * 4 + 4)):
                    for h in range(2):
                        nc.tensor.transpose(
                            out=tp[h][:, q * 128:(q + 1) * 128],
                            in_=gts[t][:, h * 128:(h + 1) * 128],
                            identity=identb[:],
                        )
                nc.vector.tensor_copy(
                    out=g_sb[0][:, nch * 512:(nch + 1) * 512], in_=tp[0][:])
                nc.scalar.copy(
                    out=g_sb[1][:, nch * 512:(nch + 1) * 512], in_=tp[1][:])

            # ---- Z accumulation [128b, 256] over 32 K-tiles; bank0's
            # G transposes slot in at k=24 where the stream still paces
            # the (cold) PE and their inputs (decode ~12us) are ready
            z_ps = zps.tile([128, BASIS], F32, tag="z")
            for i, k in enumerate(Z_ORDER):
                nc.tensor.matmul(
                    z_ps[:],
                    lhsT=x_sb[:, k * 128:(k + 1) * 128],
                    rhs=b_sb[:, k * 256:(k + 1) * 256],
                    start=(i == 0), stop=(i == NK - 1),
                )
                if i in (3, 11, 15):
                    # chunk sems land ~0.5-1.2us after the PE drains
                    # the prior chunk (HBM receipt latency); keep the
                    # PE busy with junk matmuls so the HAM clock-gate
                    # keeps accumulating toward the 2.4GHz unthrottle
                    # (any idle window resets it; transposes don't
                    # count as activity)
                    for j in range(8 if i == 3 else 4):
                        jp = tps.tile([128, 128], F32, tag="junk",
                                      name=f"junk{i}_{j}")
                        nc.tensor.matmul(jp[:], lhsT=identb[:],
                                         rhs=identb[:],
                                         start=True, stop=True)
                if i == 7 and GT_MODE == "pe":
                    emit_gt_bank(0)

            # Z -> bf16 (halves cast in parallel on DVE+ACT),
            # PE-transpose into Z^T halves
            z_sb = pool.tile([128, BASIS], BF16)
            nc.vector.tensor_copy(out=z_sb[:, 0:128], in_=z_ps[:, 0:128])
            nc.scalar.copy(out=z_sb[:, 128:256], in_=z_ps[:, 128:256])
            zt = []
            for h in range(2):
                ztp = tps.tile([128, 128], BF16, tag=f"ztp{h}", name=f"ztp{h}")
                nc.tensor.transpose(
                    out=ztp[:], in_=z_sb[:, h * 128:(h + 1) * 128],
                    identity=identb[:],
                )
                ztt = pool.tile([128, 128], BF16, tag=f"zt{h}", name=f"zt{h}")
                if h == 0:
                    nc.vector.tensor_copy(out=ztt[:], in_=ztp[:])
                else:
                    nc.scalar.copy(out=ztt[:], in_=ztp[:])
                zt.append(ztt)

            # y = Z^T.T @ G (scale already folded into G); store each
            # 512-col bank as soon as its copy lands
            def emit_y(nch):
                y_ps = yps.tile([128, 512], F32, tag="y",
                                name=f"y_ps{nch}")
                nc.tensor.matmul(
                    y_ps[:], lhsT=zt[0][:],
                    rhs=g_sb[0][:, nch * 512:(nch + 1) * 512],
                    start=True, stop=False,
                )
                nc.tensor.matmul(
                    y_ps[:], lhsT=zt[1][:],
                    rhs=g_sb[1][:, nch * 512:(nch + 1) * 512],
                    start=False, stop=True,
                )
                y_sb = pool.tile([128, 512], FP16, tag=f"ysb{nch}",
                                 name=f"y_sb{nch}")
                nc.vector.tensor_copy(out=y_sb[:, 0:256], in_=y_ps[:, 0:256])
                nc.scalar.copy(out=y_sb[:, 256:512], in_=y_ps[:, 256:512])
                if nch == 0:
                    nc.sync.dma_start(out=out_d[:, 0:512], in_=y_sb[:])
                else:
                    nc.scalar.dma_start(out=out_d[:, 512:1024], in_=y_sb[:])

            emit_y(0)
            if GT_MODE == "pe":
                emit_gt_bank(1)
            emit_y(1)

    nc.compile()
    return nc


_NC = None


def _get_nc():
    global _NC
    if _NC is None:
        _NC = build_nc()
    return _NC


def make_in_maps(x, codes, basis):
    import ml_dtypes

    x = np.ascontiguousarray(x, dtype=np.float32)
    basis = np.ascontiguousarray(basis, dtype=np.float32)
    codes = np.ascontiguousarray(codes, dtype=np.int32)
    f8 = ml_dtypes.float8_e3m4
    x_np_dt = f8 if QUANT == "fp8" else np.float16
    b_np_dt = np.float16 if QUANT == "fp16" else f8

    # xt[p, k*128 + m] = x[m, k*128 + p]
    xt = np.ascontiguousarray(
        (x * X_SCALE).reshape(BATCH, NK, 128).transpose(2, 1, 0)
        .reshape(128, IN_F)
    ).astype(x_np_dt)
    # bt[p, k*256 + o] = basis[o, k*128 + p]
    bt = np.ascontiguousarray(
        (basis * B_SCALE).reshape(BASIS, NK, 128).transpose(2, 1, 0)
        .reshape(128, 2 * IN_F)
    ).astype(b_np_dt)

    shared = {}
    for i, (s, e) in enumerate(X_CHUNKS):
        shared[f"xc{i}"] = np.ascontiguousarray(xt[:, s * 128:e * 128])
    for i, (s, e) in enumerate(B_CHUNKS):
        shared[f"bc{i}"] = np.ascontiguousarray(bt[:, s * 256:e * 256])

    in_maps = []
    for c in range(N_CORES):
        sh = codes[c * OPC:(c + 1) * OPC]
        # wrap-128 layout: c128[p, t] = codes[t*128 + p]
        c128 = np.ascontiguousarray(sh.reshape(NT, 128).T)
        in_maps.append({**shared, "c128": c128})
    return in_maps


def assemble_output(results):
    return np.concatenate(
        [results[c]["out"].astype(np.float32) for c in range(N_CORES)], axis=1
    )


def kernel(x, codes, basis):
    nc = _get_nc()
    in_maps = make_in_maps(x, codes, basis)
    res = run_bass_kernel_spmd(nc, in_maps, list(range(N_CORES)))
    return assemble_output(res.results)


if __name__ == "__main__":
    rng = np.random.default_rng(0)
    x = rng.standard_normal((BATCH, IN_F), dtype=np.float32)
    basis = (rng.standard_normal((BASIS, IN_F)) * 0.02).astype(np.float32)
    codes = rng.integers(0, 1 << 22, size=(OUT_F,), dtype=np.int32)
    y = kernel(x, codes, basis)

    idx = codes & 255
    r = ((codes >> 8) & 4095).astype(np.float32) / R_LEVELS
    sign = np.where(((codes >> 20) & 1) == 1, -1.0, 1.0).astype(np.float32)
    scale = sign * np.tanh(r)
    W = scale[:, None] * basis[idx]
    y_ref = x @ W.T
    err = np.linalg.norm(y - y_ref) / np.linalg.norm(y_ref)
    print("rel err:", err)


# revision 26
# speedup vs baseline: 1.0226x; 1.0226x over previous
"""Trainium2 Bass kernel for nn_BitfieldLinear (vq_codebook).

Reference computation:
    idx   = codes & 0xFF            (basis row, 256 entries)
    r_q   = (codes >> 8) & 0xFFF
    sign  = bit20 ? -1 : +1
    scale = sign * tanh(r_q / 4095)
    W     = scale[:, None] * basis[idx]        # [8192, 4096]
    y     = x @ W.T                            # [128, 8192]

Key factorization (never materialize the 128MB W):
    Z = x @ basis.T                            # [128, 256]  tiny matmul
    y[b, j] = scale[j] * Z[b, idx[j]]          # column gather + scale

The gather+scale is a matmul with a scaled one-hot matrix:
    G[k, j] = scale[j] * (idx[j] == k)         # [256, 1024] per core
    y_core  = Z @ G                            # [128, 1024]

Sharding: out_features column-parallel across 8 cores (1024 codes per
core); x and basis replicated (8-core collectives are ~68us on this
harness -- far slower than recomputing Z per core).

v4 layout (from v2/v3 trace analysis):
  - 9 input DMAs on the two HWDGE queues only (c128 first), sized so
    the last chunks are small: SDMA round-robins between queues at
    packet granularity, so all in-flight transfers complete near the
    stream end -- big early chunks + small late chunks minimize the
    Z-tail wait.
  - Z accumulation is commutative: k-tiles are consumed in DMA-arrival
    order, not index order.
  - PE HAM warmup (~16 dummy matmuls on a memset scratch): the PE clock
    gate needs ~3.4us of sustained activity before it doubles to
    2.4GHz; v3 warmed too briefly and ran all of Z at half clock.
  - G^T tiles (partition=code, scale fused into the is_eq build) are
    transposed into matmul layout by 16 xbar DMA-transposes on the idle
    HWDGE queues -- zero PE cost vs 16 PE transposes (BITF_GT=pe
    fallback keeps the old path).
  - copies split DVE/ACT (the ACT table load overlaps the stream).
QUANT="fp8" streams x/basis as fp8e3m4 (pre-scaled by 2/64, compensated
in the tanh coeffs): 1.5MB/core stream at ~1.4% rel err.
"""

import os
import sys

for _p in ("/opt/trn_rl_repo", "/opt/pypackages"):
    if _p not in sys.path:
        sys.path.insert(0, _p)

import numpy as np

import concourse.bacc as bacc
import concourse.mybir as mybir
import concourse.tile as tile
from concourse.alu_op_type import AluOpType
from concourse.bass_utils import run_bass_kernel_spmd

N_CORES = 8
BATCH = 128
IN_F = 4096
OUT_F = 8192
BASIS = 256
OPC = OUT_F // N_CORES      # 1024 output columns per core
NK = IN_F // 128            # 32 K-tiles
NT = OPC // 128             # 8 code-tiles per core
R_LEVELS = 4095.0

F32 = mybir.dt.float32
BF16 = mybir.dt.bfloat16
FP16 = mybir.dt.float16
FP8 = mybir.dt.float8e3
I32 = mybir.dt.int32

QUANT = os.environ.get("BITF_QUANT", "fp8")
GT_MODE = os.environ.get("BITF_GT", "pe")    # 'dma' xbar | 'pe' transpose
N_WARM = int(os.environ.get("BITF_WARM", "0"))
X_DT = FP8 if QUANT == "fp8" else FP16
B_DT = FP16 if QUANT == "fp16" else FP8
X_SCALE = 2.0 if QUANT == "fp8" else 1.0     # keep fp8e3m4 out of denormals
B_SCALE = 64.0 if QUANT in ("fp8", "fp8b") else 1.0
_COMP = 1.0 / (X_SCALE * B_SCALE)            # folded into tanh coeffs

# tanh(r) ~= r*(c0 + c1 u + c2 u^2 + c3 u^3), u=r^2, r in [0,1]
# (max rel err 8e-5, negligible vs the fp8 input error); coeffs carry
# the fp8 pre-scale compensation
TANH_C = [c * _COMP for c in (
    9.9991860534e-01, -3.3065536868e-01, 1.1890093882e-01,
    -2.6632289374e-02)]

# input chunks (k-tile ranges).  DMA completion sems fire ~1.5-2.3us
# after the data lands (HBM read receipt under load), so the first
# chunks are small to let Z start early, x is fully front-loaded on
# sync, and basis streams in 128KB chunks on scalar for a smooth
# availability ramp (v6's single k4-16 x chunk left the PE idle 2.3us,
# which also reset the HAM clock-gate warmup).
X_CHUNKS = [(0, 2), (2, 8), (8, 20), (20, 32)]       # 32+96+192+192 on sync
B_CHUNKS = [(0, 2), (2, 4), (4, 6), (6, 8), (8, 12), (12, 16),
            (16, 20), (20, 24), (24, 28), (28, 32)]
SYNC_ORDER = ["c128", "xc0", "xc1", "xc2", "xc3"]
GP_ORDER = ["bc8", "bc9"]
SCAL_ORDER = ["bc0", "bc1", "bc2", "bc3", "bc4", "bc5", "bc6", "bc7"]
# basis arrives in ascending k on scalar at ~the cold-PE consumption
# rate; x (half the bytes per k-tile) stays ahead on sync
Z_ORDER = list(range(NK))


def build_nc():
    nc = bacc.Bacc(
        "TRN2",
        target_bir_lowering=False,
        debug=False,
        num_devices=N_CORES,
    )

    c128_d = nc.dram_tensor("c128", [128, NT], I32, kind="ExternalInput")
    xd = {
        f"xc{i}": nc.dram_tensor(f"xc{i}", [128, (e - s) * 128], X_DT,
                                 kind="ExternalInput")
        for i, (s, e) in enumerate(X_CHUNKS)
    }
    bd = {
        f"bc{i}": nc.dram_tensor(f"bc{i}", [128, (e - s) * 256], B_DT,
                                 kind="ExternalInput")
        for i, (s, e) in enumerate(B_CHUNKS)
    }
    out_d = nc.dram_tensor("out", [128, OPC], FP16, kind="ExternalOutput")

    with tile.TileContext(nc) as tc:
        with (
            tc.tile_pool(name="pool", bufs=1) as pool,
            tc.tile_pool(name="zps", bufs=1, space="PSUM") as zps,
            tc.tile_pool(name="tps", bufs=1, space="PSUM") as tps,
            tc.tile_pool(name="yps", bufs=2, space="PSUM") as yps,
        ):
            # (HAM warmup removed: measured on v3/v4, dummy matmuls
            # advance the clock-gate no faster than real Z matmuls do,
            # while delaying Z's start by their own duration)
            if N_WARM:
                scr = pool.tile([128, 256], BF16)
                nc.gpsimd.memset(scr[:], 0.0)
                for w in range(N_WARM):
                    wp = tps.tile([128, 256], F32, tag="warm",
                                  name=f"warm{w}")
                    nc.tensor.matmul(
                        wp[:], lhsT=scr[:, 0:128], rhs=scr[:],
                        start=True, stop=True,
                    )

            # ---- input DMAs: c128 on the otherwise-idle SWDGE queue
            # (so sync's first data chunk dispatches at t0), stream
            # chunks in k-order interleaved across the two HWDGE queues
            c128 = pool.tile([128, NT], I32)
            x_sb = pool.tile([128, IN_F], X_DT)
            b_sb = pool.tile([128, 2 * IN_F], B_DT)
            def issue(name):
                if name in GP_ORDER:
                    q = nc.gpsimd
                elif name in SYNC_ORDER:
                    q = nc.sync
                else:
                    q = nc.scalar
                if name == "c128":
                    q.dma_start(out=c128[:], in_=c128_d[:])
                elif name.startswith("xc"):
                    s, e = X_CHUNKS[int(name[2:])]
                    q.dma_start(out=x_sb[:, s * 128:e * 128], in_=xd[name][:])
                else:
                    s, e = B_CHUNKS[int(name[2:])]
                    q.dma_start(out=b_sb[:, s * 256:e * 256], in_=bd[name][:])

            for name in ["c128", "xc0", "bc0", "bc1", "xc1", "bc2", "bc3",
                         "xc2", "bc4", "bc5", "xc3", "bc6", "bc7"]:
                issue(name)

            # ---- constants: iota row [0..255], partition iota, identity
            iota_row_i = pool.tile([128, BASIS], I32)
            nc.gpsimd.iota(out=iota_row_i[:], pattern=[[1, BASIS]], base=0,
                           channel_multiplier=0)
            iota_part_i = pool.tile([128, 1], I32)
            nc.gpsimd.iota(out=iota_part_i[:], pattern=[[1, 1]], base=0,
                           channel_multiplier=1)
            # tail basis chunks ride SWDGE (issued after the iotas):
            # its completion receipt (~2us) undercuts the loaded-HWDGE
            # receipt (~3.5us) that was gating Z's last k-tiles
            for name in GP_ORDER:
                issue(name)

            # iota row in bf16: integer values <=255 are exact in bf16
            # and 2-byte in0 doubles DVE throughput on the G builds
            iota_b = pool.tile([128, BASIS], BF16)
            nc.vector.tensor_scalar_mul(out=iota_b[:], in0=iota_row_i[:],
                                        scalar1=1.0)
            iota_part_f = pool.tile([128, 1], F32)
            nc.vector.tensor_scalar_mul(out=iota_part_f[:],
                                        in0=iota_part_i[:], scalar1=1.0)
            identb = pool.tile([128, 128], BF16)
            nc.vector.tensor_scalar(
                out=identb[:], in0=iota_b[:, 0:128],
                scalar1=iota_part_f[:, 0:1], scalar2=None,
                op0=AluOpType.is_equal,
            )

            # ---- decode codes -> idx_f (f32), scl (f32), both [128, NT]
            idx_f = pool.tile([128, NT], F32)
            scl = pool.tile([128, NT], F32)

            idx_i = pool.tile([128, NT], I32, name="idx_i")
            nc.vector.tensor_scalar(
                out=idx_i[:], in0=c128[:],
                scalar1=255, scalar2=None, op0=AluOpType.bitwise_and,
            )
            nc.vector.tensor_scalar_mul(out=idx_f[:], in0=idx_i[:],
                                        scalar1=1.0)
            rq_i = pool.tile([128, NT], I32, name="rq_i")
            nc.vector.tensor_scalar(
                out=rq_i[:], in0=c128[:],
                scalar1=8, scalar2=4095,
                op0=AluOpType.logical_shift_right,
                op1=AluOpType.bitwise_and,
            )
            r = pool.tile([128, NT], F32, name="r")
            nc.vector.tensor_scalar_mul(out=r[:], in0=rq_i[:],
                                        scalar1=1.0 / R_LEVELS)
            u = pool.tile([128, NT], F32, name="u")
            nc.vector.tensor_tensor(out=u[:], in0=r[:], in1=r[:],
                                    op=AluOpType.mult)
            p = pool.tile([128, NT], F32, name="p")
            nc.vector.tensor_scalar(
                out=p[:], in0=u[:], scalar1=TANH_C[3], scalar2=TANH_C[2],
                op0=AluOpType.mult, op1=AluOpType.add,
            )
            for ci_ in (1, 0):
                nc.vector.tensor_tensor(out=p[:], in0=p[:], in1=u[:],
                                        op=AluOpType.mult)
                nc.vector.tensor_scalar(
                    out=p[:], in0=p[:], scalar1=TANH_C[ci_], scalar2=None,
                    op0=AluOpType.add,
                )
            th = pool.tile([128, NT], F32, name="th")
            nc.vector.tensor_tensor(out=th[:], in0=p[:], in1=r[:],
                                    op=AluOpType.mult)
            sg_i = pool.tile([128, NT], I32, name="sg_i")
            nc.vector.tensor_scalar(
                out=sg_i[:], in0=c128[:],
                scalar1=20, scalar2=1,
                op0=AluOpType.logical_shift_right,
                op1=AluOpType.bitwise_and,
            )
            sgn = pool.tile([128, NT], F32, name="sgn")
            nc.vector.tensor_scalar(
                out=sgn[:], in0=sg_i[:],
                scalar1=-2.0, scalar2=1.0,
                op0=AluOpType.mult, op1=AluOpType.add,
            )
            nc.vector.tensor_tensor(out=scl[:], in0=th[:], in1=sgn[:],
                                    op=AluOpType.mult)

            # ---- G^T tiles (bf16): gt[p, k] = scl[t*128+p] * (idx==k)
            gts = []
            for t in range(NT):
                gt = pool.tile([128, BASIS], BF16, tag=f"gt{t}",
                               name=f"gt{t}")
                nc.vector.tensor_scalar(
                    out=gt[:], in0=iota_b[:],
                    scalar1=idx_f[:, t:t + 1], scalar2=scl[:, t:t + 1],
                    op0=AluOpType.is_equal, op1=AluOpType.mult,
                )
                gts.append(gt)

            # ---- G in matmul layout [basis-k, code]: xbar DMA
            # transposes on the (by now idle) HWDGE queues, zero PE cost
            g_sb = [pool.tile([128, OPC], BF16, tag=f"g{h}", name=f"g_sb{h}")
                    for h in range(2)]
            if GT_MODE == "dma":
                for t in range(NT):
                    for h in range(2):
                        q = nc.sync if (t * 2 + h) % 2 == 0 else nc.scalar
                        q.dma_start(
                            out=g_sb[h][:, t * 128:(t + 1) * 128],
                            in_=gts[t][:, h * 128:(h + 1) * 128],
                            transpose=True,
                        )

            # The 8 G transposes of a bank write quarters of two shared
            # [128, 512] PSUM tiles so ONE wide copy per half moves them
            # to SBUF (v5/v6's per-tile copies gated the transposes at
            # ~650ns each).
            def emit_gt_bank(nch):
                tp = [tps.tile([128, 512], BF16, tag=f"gtp{h}",
                               name=f"gtp{h}_{nch}") for h in range(2)]
                for q, t in enumerate(range(nch * 4, nch * 4 + 4)):
                    for h in range(2):
                        nc.tensor.transpose(
                            out=tp[h][:, q * 128:(q + 1) * 128],
                            in_=gts[t][:, h * 128:(h + 1) * 128],
                            identity=identb[:],
                        )
                nc.vector.tensor_copy(
                    out=g_sb[0][:, nch * 512:(nch + 1) * 512], in_=tp[0][:])
                nc.scalar.copy(
                    out=g_sb[1][:, nch * 512:(nch + 1) * 512], in_=tp[1][:])

            # ---- Z accumulation [128b, 256] over 32 K-tiles; bank0's
            # G transposes slot in at k=24 where the stream still paces
            # the (cold) PE and their inputs (decode ~12us) are ready
            z_ps = zps.tile([128, BASIS], F32, tag="z")
            for i, k in enumerate(Z_ORDER):
                nc.tensor.matmul(
                    z_ps[:],
                    lhsT=x_sb[:, k * 128:(k + 1) * 128],
                    rhs=b_sb[:, k * 256:(k + 1) * 256],
                    start=(i == 0), stop=(i == NK - 1),
                )
                if i in (3, 11, 15):
                    # chunk sems land ~0.5-1.2us after the PE drains
                    # the prior chunk (HBM receipt latency); keep the
                    # PE busy with junk matmuls so the HAM clock-gate
                    # keeps accumulating toward the 2.4GHz unthrottle
                    # (any idle window resets it; transposes don't
                    # count as activity)
                    for j in range(8 if i == 3 else 4):
                        jp = tps.tile([128, 128], F32, tag="junk",
                                      name=f"junk{i}_{j}")
                        nc.tensor.matmul(jp[:], lhsT=identb[:],
                                         rhs=identb[:],
                                         start=True, stop=True)
                if i == 7 and GT_MODE == "pe":
                    emit_gt_bank(0)

            # Z -> bf16 (halves cast in parallel on DVE+ACT),
            # PE-transpose into Z^T halves
            z_sb = pool.tile([128, BASIS], BF16)
            nc.vector.tensor_copy(out=z_sb[:, 0:128], in_=z_ps[:, 0:128])
            nc.scalar.copy(out=z_sb[:, 128:256], in_=z_ps[:, 128:256])
            zt = []
            for h in range(2):
                ztp = tps.tile([128, 128], BF16, tag=f"ztp{h}", name=f"ztp{h}")
                nc.tensor.transpose(
                    out=ztp[:], in_=z_sb[:, h * 128:(h + 1) * 128],
                    identity=identb[:],
                )
                ztt = pool.tile([128, 128], BF16, tag=f"zt{h}", name=f"zt{h}")
                if h == 0:
                    nc.vector.tensor_copy(out=ztt[:], in_=ztp[:])
                else:
                    nc.scalar.copy(out=ztt[:], in_=ztp[:])
                zt.append(ztt)

            # y = Z^T.T @ G (scale already folded into G); store each
            # 512-col bank as soon as its copy lands
            def emit_y(nch):
                y_ps = yps.tile([128, 512], F32, tag="y",
                                name=f"y_ps{nch}")
                nc.tensor.matmul(
                    y_ps[:], lhsT=zt[0][:],
                    rhs=g_sb[0][:, nch * 512:(nch + 1) * 512],
                    start=True, stop=False,
                )
                nc.tensor.matmul(
                    y_ps[:], lhsT=zt[1][:],
                    rhs=g_sb[1][:, nch * 512:(nch + 1) * 512],
                    start=False, stop=True,
                )
                y_sb = pool.tile([128, 512], FP16, tag=f"ysb{nch}",
                                 name=f"y_sb{nch}")
                nc.vector.tensor_copy(out=y_sb[:, 0:256], in_=y_ps[:, 0:256])
                nc.scalar.copy(out=y_sb[:, 256:512], in_=y_ps[:, 256:512])
                if nch == 0:
                    nc.sync.dma_start(out=out_d[:, 0:512], in_=y_sb[:])
                else:
                    nc.scalar.dma_start(out=out_d[:, 512:1024], in_=y_sb[:])

            emit_y(0)
            if GT_MODE == "pe":
                emit_gt_bank(1)
            emit_y(1)

    nc.compile()
    return nc


_NC = None


def _get_nc():
    global _NC
    if _NC is None:
        _NC = build_nc()
    return _NC


def make_in_maps(x, codes, basis):
    import ml_dtypes

    x = np.ascontiguousarray(x, dtype=np.float32)
    basis = np.ascontiguousarray(basis, dtype=np.float32)
    codes = np.ascontiguousarray(codes, dtype=np.int32)
    f8 = ml_dtypes.float8_e3m4
    x_np_dt = f8 if QUANT == "fp8" else np.float16
    b_np_dt = np.float16 if QUANT == "fp16" else f8

    # xt[p, k*128 + m] = x[m, k*128 + p]
    xt = np.ascontiguousarray(
        (x * X_SCALE).reshape(BATCH, NK, 128).transpose(2, 1, 0)
        .reshape(128, IN_F)
    ).astype(x_np_dt)
    # bt[p, k*256 + o] = basis[o, k*128 + p]
    bt = np.ascontiguousarray(
        (basis * B_SCALE).reshape(BASIS, NK, 128).transpose(2, 1, 0)
        .reshape(128, 2 * IN_F)
    ).astype(b_np_dt)

    shared = {}
    for i, (s, e) in enumerate(X_CHUNKS):
        shared[f"xc{i}"] = np.ascontiguousarray(xt[:, s * 128:e * 128])
    for i, (s, e) in enumerate(B_CHUNKS):
        shared[f"bc{i}"] = np.ascontiguousarray(bt[:, s * 256:e * 256])

    in_maps = []
    for c in range(N_CORES):
        sh = codes[c * OPC:(c + 1) * OPC]
        # wrap-128 layout: c128[p, t] = codes[t*128 + p]
        c128 = np.ascontiguousarray(sh.reshape(NT, 128).T)
        in_maps.append({**shared, "c128": c128})
    return in_maps


def assemble_output(results):
    return np.concatenate(
        [results[c]["out"].astype(np.float32) for c in range(N_CORES)], axis=1
    )


def kernel(x, codes, basis):
    nc = _get_nc()
    in_maps = make_in_maps(x, codes, basis)
    res = run_bass_kernel_spmd(nc, in_maps, list(range(N_CORES)))
    return assemble_output(res.results)


if __name__ == "__main__":
    rng = np.random.default_rng(0)
    x = rng.standard_normal((BATCH, IN_F), dtype=np.float32)
    basis = (rng.standard_normal((BASIS, IN_F)) * 0.02).astype(np.float32)
    codes = rng.integers(0, 1 << 22, size=(OUT_F,), dtype=np.int32)
    y = kernel(x, codes, basis)

    idx = codes & 255
    r = ((codes >> 8) & 4095).astype(np.float32) / R_LEVELS
    sign = np.where(((codes >> 20) & 1) == 1, -1.0, 1.0).astype(np.float32)
    scale = sign * np.tanh(r)
    W = scale[:, None] * basis[idx]
    y_ref = x @ W.T
    err = np.linalg.norm(y - y_ref) / np.linalg.norm(y_ref)
    print("rel err:", err)
